# revision 11
# baseline (speedup 1.0000x reference)
"""Trainium2 Bass kernel for nn_Cross_MultiAttention (8-head cross attention).

Sharding: one attention head per NeuronCore (8 heads / 8 cores).

Host folds the shared 1x1 input conv into each head's q/k/v projections
(Aq = wq_h @ w_in etc.), so each core:
  - projects q/k/v for its head directly from (x+pos) / (context+pos) in bf16,
  - splits q and k into hi/lo fp8 pairs and packs them into the contraction
    slots of a DoubleRow fp8 matmul: 128 partitions = 4 blocks of 32 depth
    dims carrying (qh,qh,ql,ql) x (kh,kl,kh,kl), 2 identical planes, so one
    DR matmul computes 2*(qh+ql).(kh+kl) = 2*q.k at 2x fp16 speed with
    ~fp16 accuracy. Scores stay TRANSPOSED (keys on partitions).
  - exp runs on the scalar engine (scale 1/32, max-free since |s/16| < ~2.5)
    writing bf16 probabilities; masking is a bitwise AND against 0xFFFF/0x0000
    u16 words (DVE 2x mode), streamed through a small rotating SBUF window.
  - P@V runs in bf16 with an appended ones-column on V so the softmax
    denominator falls out of the same matmul. The head's slice of the output
    projection is applied to the UNNORMALIZED attention output; the per-query
    denominator row is exported.
Host divides each partial [256, 5000] by its denominator, sums the 8
partials, adds b_out, reshapes to [256, 50, 100].
"""

import numpy as np

import concourse.bacc as bacc
import concourse.tile as tile
import concourse.mybir as mybir
from concourse.bass_utils import run_bass_kernel_spmd

F32 = mybir.dt.float32
BF16 = mybir.dt.bfloat16
F8 = mybir.dt.float8e4
U16 = mybir.dt.uint16
AF = mybir.ActivationFunctionType
ALU = mybir.AluOpType
DR = mybir.MatmulPerfMode.DoubleRow

EMB = 256
HEADS = 8
DEPTH = 32
IN_CH = 256
H, W = 50, 100
N_TOK = H * W            # 5000
N_PAD = 5120             # key dim padded to a multiple of 128 (and 16)
SCALE = EMB ** (-0.5)    # 1/16

WSZ = 1024               # query stripe width
JSZ = 128                # key tile (partition dim of transposed scores)
NJ = 40                  # N_PAD / 128; key tile 39 has only 8 real keys
MQ = 10                  # j-tiles per mask DMA (quarter stripe)
MWIN = 3                 # rotating mask-window depth (quarters)


def _tiles(total, size):
    out = []
    p = 0
    while p < total:
        out.append((p, min(size, total - p)))
        p += size
    return out


def build_nc(n_tok=N_TOK, num_devices=8):
    nc = bacc.Bacc("TRN2", target_bir_lowering=False, debug=False,
                   num_devices=num_devices)

    D = DEPTH
    xp_d = nc.dram_tensor("xp", (IN_CH, n_tok), BF16, kind="ExternalInput").ap()
    cp_d = nc.dram_tensor("cp", (IN_CH, n_tok), BF16, kind="ExternalInput").ap()
    # mask words, key-tile-major: [j % 128, j // 128, i], 0xFFFF keep / 0 drop
    nmT_d = nc.dram_tensor("nmT", (JSZ, NJ, n_tok), U16, kind="ExternalInput").ap()
    AqT_d = nc.dram_tensor("AqT", (IN_CH, 4 * D), BF16, kind="ExternalInput").ap()
    cq_d = nc.dram_tensor("cq", (4 * D, 1), F32, kind="ExternalInput").ap()
    AkT_d = nc.dram_tensor("AkT", (IN_CH, 4 * D), BF16, kind="ExternalInput").ap()
    ck_d = nc.dram_tensor("ck", (4 * D, 1), F32, kind="ExternalInput").ap()
    AvT_d = nc.dram_tensor("AvT", (IN_CH, D), BF16, kind="ExternalInput").ap()
    cvb_d = nc.dram_tensor("cvb", (128, D), F32, kind="ExternalInput").ap()
    woT_d = nc.dram_tensor("woT", (D, EMB), BF16, kind="ExternalInput").ap()
    y_d = nc.dram_tensor("y", (EMB, n_tok), F32, kind="ExternalOutput").ap()
    dn_d = nc.dram_tensor("dn", (1, n_tok), F32, kind="ExternalOutput").ap()

    ntiles = _tiles(n_tok, 512)   # 512-wide tiles (projections)
    wtiles = _tiles(n_tok, WSZ)   # query stripes
    NW = len(wtiles)

    with tile.TileContext(nc) as tc:
        with (
            tc.tile_pool(name="persist", bufs=1) as persist,
            tc.tile_pool(name="consts", bufs=1) as consts,
        ):
            # ---- constants to SBUF ----
            AqT_sb = consts.tile([128, 2, 4 * D], BF16)
            AkT_sb = consts.tile([128, 2, 4 * D], BF16)
            AvT_sb = consts.tile([128, 2, D], BF16)
            for ct in range(2):
                nc.sync.dma_start(AqT_sb[:, ct, :], AqT_d[ct * 128:(ct + 1) * 128, :])
                nc.sync.dma_start(AkT_sb[:, ct, :], AkT_d[ct * 128:(ct + 1) * 128, :])
                nc.sync.dma_start(AvT_sb[:, ct, :], AvT_d[ct * 128:(ct + 1) * 128, :])
            cq_sb = consts.tile([4 * D, 1], F32)
            nc.sync.dma_start(cq_sb[:, :], cq_d[:, :])
            ck_sb = consts.tile([4 * D, 1], F32)
            nc.sync.dma_start(ck_sb[:, :], ck_d[:, :])
            cvb_sb = consts.tile([128, D], F32)
            nc.sync.dma_start(cvb_sb[:, :], cvb_d[:, :])
            woT_sb = consts.tile([D, EMB], BF16)
            nc.sync.dma_start(woT_sb[:, :], woT_d[:, :])

            # ---- persistent activations ----
            qT8 = persist.tile([128, 2, N_PAD], F8)
            kT8 = persist.tile([128, 2, N_PAD], F8)
            # v: [j mod 128, j // 128, 0:32 v | 32 ones]
            v_sb = persist.tile([128, NJ, D + 1], BF16)
            # bf16 probabilities for one whole stripe; u16 view for mask AND
            p_store = persist.tile([128, NJ, WSZ], BF16)
            p16 = p_store.bitcast(U16)
            # rotating mask window (quarter-stripe granularity)
            m_sb = persist.tile([128, MWIN, MQ, WSZ], U16)

            nc.any.memset(v_sb[:, :, :], 0.0)
            nc.any.memset(v_sb[:, :, D:D + 1], 1.0)
            # zero k for padded keys 5000..5119 (scores 0, masked to 0)
            nc.any.memset(kT8[:, :, n_tok:N_PAD], 0.0)

            # ---- stage 1: project q/k/v from (x|context)+pos, split hi/lo --
            with (
                tc.tile_pool(name="stage1", bufs=1) as stage1,
                tc.tile_pool(name="proj_in", bufs=3) as proj_in,
                tc.tile_pool(name="qk_ps", bufs=2, space="PSUM") as qk_ps,
                tc.tile_pool(name="v_ps", bufs=2, space="PSUM") as v_ps,
            ):
                q_hi = stage1.tile([128, N_PAD], F8)
                q_lo = stage1.tile([128, N_PAD], F8)
                k_hi = stage1.tile([128, N_PAD], F8)
                k_lo = stage1.tile([128, N_PAD], F8)

                for (n0, ns) in ntiles:
                    img_t = proj_in.tile([128, 2, 512], BF16, name="img_t")
                    for ct in range(2):
                        nc.sync.dma_start(
                            img_t[:, ct, :ns],
                            xp_d[ct * 128:(ct + 1) * 128, n0:n0 + ns])
                    qps = qk_ps.tile([4 * D, 512], F32, name="qps")
                    for ct in range(2):
                        nc.tensor.matmul(qps[:, :ns], AqT_sb[:, ct, :],
                                         img_t[:, ct, :ns],
                                         start=(ct == 0), stop=(ct == 1))
                    # hi = fp8(q + bias) on ACT; lo = (q + bias) - hi on DVE
                    nc.scalar.activation(q_hi[:, n0:n0 + ns], qps[:, :ns],
                                         AF.Identity, bias=cq_sb[:, :])
                    nc.vector.scalar_tensor_tensor(
                        q_lo[:, n0:n0 + ns], qps[:, :ns], cq_sb[:, :],
                        q_hi[:, n0:n0 + ns], op0=ALU.add, op1=ALU.subtract)

                for (n0, ns) in ntiles:
                    img_t = proj_in.tile([128, 2, 512], BF16, name="img_t")
                    for ct in range(2):
                        nc.sync.dma_start(
                            img_t[:, ct, :ns],
                            cp_d[ct * 128:(ct + 1) * 128, n0:n0 + ns])
                    kps = qk_ps.tile([4 * D, 512], F32, name="qps")
                    for ct in range(2):
                        nc.tensor.matmul(kps[:, :ns], AkT_sb[:, ct, :],
                                         img_t[:, ct, :ns],
                                         start=(ct == 0), stop=(ct == 1))
                    nc.scalar.activation(k_hi[:, n0:n0 + ns], kps[:, :ns],
                                         AF.Identity, bias=ck_sb[:, :])
                    nc.vector.scalar_tensor_tensor(
                        k_lo[:, n0:n0 + ns], kps[:, :ns], ck_sb[:, :],
                        k_hi[:, n0:n0 + ns], op0=ALU.add, op1=ALU.subtract)
                    # v projection for the j-tiles inside this 512 stripe
                    for (jj0, jjs) in _tiles(ns, JSZ):
                        jt = (n0 + jj0) // JSZ
                        vps = v_ps.tile([128, D], F32, name="vps")
                        for ct in range(2):
                            nc.tensor.matmul(
                                vps[:jjs, :],
                                img_t[:, ct, jj0:jj0 + jjs],
                                AvT_sb[:, ct, :],
                                start=(ct == 0), stop=(ct == 1))
                        nc.vector.tensor_add(
                            v_sb[:jjs, jt, 0:D], vps[:jjs, :], cvb_sb[:jjs, :])

                # assemble DoubleRow operands (SBUF->SBUF DMAs):
                # q blocks (qh, qh, ql, ql); k blocks (kh, kl, kh, kl);
                # both planes identical.
                for t in range(2):
                    nc.sync.dma_start(qT8[0:64, t, :n_tok], q_hi[0:64, :n_tok])
                    nc.sync.dma_start(qT8[64:128, t, :n_tok], q_lo[64:128, :n_tok])
                    nc.sync.dma_start(kT8[0:32, t, :n_tok], k_hi[0:32, :n_tok])
                    nc.sync.dma_start(kT8[32:64, t, :n_tok], k_lo[32:64, :n_tok])
                    nc.sync.dma_start(kT8[64:96, t, :n_tok], k_hi[64:96, :n_tok])
                    nc.sync.dma_start(kT8[96:128, t, :n_tok], k_lo[96:128, :n_tok])

            # ---- stage 2: pipelined attention + output projection ----
            NQ = NJ // MQ  # mask quarters per stripe
            gseq = [(w, q) for w in range(NW) for q in range(NQ)]

            def load_mask(g):
                w, q = gseq[g]
                i0, isz = wtiles[w]
                nc.sync.dma_start(m_sb[:, g % MWIN, :, :isz],
                                  nmT_d[:, q * MQ:(q + 1) * MQ, i0:i0 + isz])

            with (
                tc.tile_pool(name="s_ps", bufs=2, space="PSUM") as s_ps_pool,
                tc.tile_pool(name="av_ps", bufs=1, space="PSUM") as av_ps_pool,
                tc.tile_pool(name="y_ps", bufs=2, space="PSUM") as y_ps_pool,
                tc.tile_pool(name="out_sb", bufs=2) as out_pool,
                tc.tile_pool(name="ysb", bufs=4) as ysb_pool,
            ):
                def emit_pv(av, jt, isz):
                    for (h0, hs) in _tiles(isz, 512):
                        nc.tensor.matmul(
                            av[:, h0:h0 + hs], v_sb[:, jt, :],
                            p_store[:, jt, h0:h0 + hs],
                            start=(jt == 0), stop=(jt == NJ - 1))

                def epilogue(av, i0p, iszp):
                    # unnormalized head output + denominator row; partial
                    # output projection for a finished stripe
                    unn = out_pool.tile([D + 1, WSZ], BF16, name="unn")
                    nc.vector.tensor_copy(unn[:, :iszp], av[0:D + 1, :iszp])
                    dnt = out_pool.tile([1, WSZ], F32, name="dnt")
                    nc.vector.tensor_copy(dnt[:, :iszp], av[D:D + 1, :iszp])
                    nc.sync.dma_start(dn_d[:, i0p:i0p + iszp], dnt[:, :iszp])
                    for c2 in range(2):
                        for (h0, hs) in _tiles(iszp, 512):
                            yps = y_ps_pool.tile([128, 512], F32, name="yps")
                            nc.tensor.matmul(
                                yps[:, :hs],
                                woT_sb[:, c2 * 128:(c2 + 1) * 128],
                                unn[0:D, h0:h0 + hs],
                                start=True, stop=True)
                            ysb = ysb_pool.tile([128, 512], F32, name="ysb")
                            nc.vector.tensor_copy(ysb[:, :hs], yps[:, :hs])
                            nc.sync.dma_start(
                                y_d[c2 * 128:(c2 + 1) * 128,
                                    i0p + h0:i0p + h0 + hs],
                                ysb[:, :hs])

                load_mask(0)
                load_mask(1)
                pending = None
                for w, (i0, isz) in enumerate(wtiles):
                    av = av_ps_pool.tile([D + 1, WSZ], F32, name="av")
                    done_pv = 0
                    for jt in range(NJ):
                        g = w * NQ + jt // MQ
                        if jt % MQ == 0 and g + 2 < len(gseq):
                            load_mask(g + 2)
                        s = s_ps_pool.tile([128, WSZ], F32, name="s")
                        for (h0, hs) in _tiles(isz, 512):
                            nc.tensor.matmul(
                                s[:, h0:h0 + hs],
                                kT8[:, :, jt * JSZ:(jt + 1) * JSZ],
                                qT8[:, :, i0 + h0:i0 + h0 + hs],
                                start=True, stop=True, perf_mode=DR)
                        nc.scalar.activation(
                            p_store[:, jt, :isz], s[:, :isz],
                            AF.Exp, scale=float(SCALE) / 2.0)
                        nc.vector.tensor_tensor(
                            p16[:, jt, :isz], p16[:, jt, :isz],
                            m_sb[:, g % MWIN, jt % MQ, :isz],
                            op=ALU.bitwise_and)
                        if jt == 2 and pending is not None:
                            epilogue(*pending)
                            pending = None
                        while done_pv < NJ and done_pv <= jt - 4:
                            emit_pv(av, done_pv, isz)
                            done_pv += 1
                    while done_pv < NJ:
                        emit_pv(av, done_pv, isz)
                        done_pv += 1
                    pending = (av, i0, isz)
                if pending is not None:
                    epilogue(*pending)

    nc.compile()
    return nc


def make_pos(row_embed, col_embed):
    """[EMB, H*W]; first half col embeds, second half row embeds."""
    d2 = row_embed.shape[1]
    pos = np.empty((EMB, H, W), np.float32)
    pos[:d2] = col_embed[:W].T[:, None, :]      # [d2, 1, W] -> broadcast H
    pos[d2:] = row_embed[:H].T[:, :, None]      # [d2, H, 1] -> broadcast W
    return pos.reshape(EMB, H * W)


def make_in_maps(x, context, pad_mask, row_embed, col_embed, w_in, b_in,
                 wq, bq, wk, bk, wv, bv, w_out, n_heads=HEADS):
    import ml_dtypes
    bf16 = ml_dtypes.bfloat16
    f8 = np.float64
    x = np.asarray(x, np.float32)
    context = np.asarray(context, np.float32)
    pad_mask = np.asarray(pad_mask)
    row_embed = np.asarray(row_embed, np.float32)
    col_embed = np.asarray(col_embed, np.float32)
    w_in = np.asarray(w_in, f8)
    b_in = np.asarray(b_in, f8)
    w_out = np.asarray(w_out, np.float32)
    wq, bq = np.asarray(wq, f8), np.asarray(bq, f8)
    wk, bk = np.asarray(wk, f8), np.asarray(bk, f8)
    wv, bv = np.asarray(wv, f8), np.asarray(bv, f8)

    pos = make_pos(row_embed, col_embed)
    xp = np.ascontiguousarray(
        (x.reshape(EMB, N_TOK) + pos).astype(bf16))
    cp = np.ascontiguousarray(
        (context.reshape(EMB, N_TOK) + pos).astype(bf16))
    # mask words: 0xFFFF keep, 0x0000 drop; key-tile-major [j%128, j//128, i];
    # padded key rows (5000..5119) all dropped
    keep = np.zeros((N_PAD, N_TOK), np.uint16)
    keep[:N_TOK] = np.where(pad_mask[0].T, 0, 0xFFFF).astype(np.uint16)
    nmT = np.ascontiguousarray(
        keep.reshape(NJ, JSZ, N_TOK).transpose(1, 0, 2))

    shared = {"xp": xp, "cp": cp, "nmT": nmT}
    in_maps = []
    for h in range(n_heads):
        sl = slice(h * DEPTH, (h + 1) * DEPTH)
        Aq = wq[sl] @ w_in          # [D, IN_CH]
        cq = wq[sl] @ b_in + bq[sl]
        Ak = wk[sl] @ w_in
        ck = wk[sl] @ b_in + bk[sl]
        Av = wv[sl] @ w_in
        cv = wv[sl] @ b_in + bv[sl]
        bf16c = lambda a: np.ascontiguousarray(a.astype(np.float32).astype(bf16))
        f32c = lambda a: np.ascontiguousarray(a.astype(np.float32))
        in_maps.append(dict(
            shared,
            AqT=bf16c(np.tile(Aq.T, (1, 4))),
            cq=f32c(np.tile(cq.reshape(DEPTH, 1), (4, 1))),
            AkT=bf16c(np.tile(Ak.T, (1, 4))),
            ck=f32c(np.tile(ck.reshape(DEPTH, 1), (4, 1))),
            AvT=bf16c(Av.T),
            cvb=f32c(np.broadcast_to(cv, (128, DEPTH))),
            woT=bf16c(w_out[:, sl].T),
        ))
    return in_maps


_CACHE = {}


def kernel(x, context, pad_mask, row_embed, col_embed, w_in, b_in,
           wq, bq, wk, bk, wv, bv, w_out, b_out):
    if "nc" not in _CACHE:
        _CACHE["nc"] = build_nc()
    nc = _CACHE["nc"]
    in_maps = make_in_maps(x, context, pad_mask, row_embed, col_embed,
                           w_in, b_in, wq, bq, wk, bk, wv, bv, w_out)
    res = run_bass_kernel_spmd(nc, in_maps, core_ids=list(range(HEADS)))
    y = np.zeros((EMB, N_TOK), np.float64)
    for c in range(HEADS):
        r = res.results[c]
        y += r["y"].astype(np.float64) / r["dn"].astype(np.float64)
    y = (y + np.asarray(b_out, np.float64)[:, None]).astype(np.float32)
    return y.reshape(EMB, H, W)
